# revision 49
# baseline (speedup 1.0000x reference)
"""Trainium2 Bass kernel for nn_BasicBlock (WeightNet/CondConv-style block).

Data parallel over batch: 32 samples -> 8 cores x 4 samples.

fp8 (e4m3) 3-pass DoubleRow conv: every conv operand (x, o1, W1, W2) is split
into fp8 hi + lo (exact residual, same scale), and each 3x3 conv accumulates
  Wh@xh + Wh@xl + Wl@xh
in fp32 PSUM via DoubleRow matmuls (contraction 256 = 2x128 chunks per
instruction).  Dropping the lo*lo term keeps bf16-level accuracy at 27/36 of
the bf16 matmul row count.  Scales: x*16, W*64 (basis pre-scaled on host),
o1*2; all folded into the BN affine host-side, so all three passes accumulate
at a single PSUM scale.

Per core, per sample (gap sums + both weight gens run one full iteration
ahead so their latency hides under the previous sample's convolutions):
  gap   = sum(x_hi, HW) @ rwT + rb      (DVE reduce / ACT accum + PE)
  a     = sigmoid(fc1p(gap) + bias)     (bias folded in as a 17th fc1 row;
          fc1 host-permuted so the broadcast of a to [128, ch, i, co] is two
          contiguous stride-0-partition DMAs through a DRAM bounce)
  W     = sum_i a_i * w2p_i             (DVE broadcast-mul chain, bf16)
  Wh/Wl = fp8 split                     (Pool copy + DVE sub)
  conv1 -> bn1+relu -> o1 hi/lo (ACT x2 + DVE sub), conv2 -> bn2 fold
  out   = relu(bn2(conv2) + x)  (ACT psum evac, Pool add+max; residual is a
          host-prepared bf16 x + bn2-bias tensor)

x hi/lo come host-pre-padded (58x58) so image loads are single contiguous
DMAs; bulk constant loads are chunked so small latency-critical DMAs never
wait long on the shared DMA engines.
"""

import sys

sys.path.insert(0, "/opt/trn_rl_repo")

import numpy as np
import ml_dtypes

import concourse.bass as bass
import concourse.tile as tile
from concourse import bacc, mybir
from concourse import bass_utils

F32 = mybir.dt.float32
BF16 = mybir.dt.bfloat16
F8 = mybir.dt.float8e4
AF = mybir.ActivationFunctionType
DR = mybir.MatmulPerfMode.DoubleRow
ALU = mybir.AluOpType

E4NP = ml_dtypes.float8_e4m3
BFNP = ml_dtypes.bfloat16

B, C, H, W = 32, 256, 56, 56
NCORES = 8
BL = B // NCORES          # samples per core
HP, WP = H + 2, W + 2     # padded 58x58
NPIX = H * W              # 3136
NPPAD = HP * WP           # 3364
NT = 7                    # h-tiles of 8 rows
TROWS = 8
NFREE = TROWS * W         # 448 columns per conv output tile
EPS = 1e-5
SX, SW, SO1 = 16.0, 64.0, 2.0


def build_program():
    nc = bacc.Bacc("TRN2", target_bir_lowering=False, debug=False,
                   num_devices=NCORES)

    xh8 = nc.dram_tensor("xh8p", [BL, C, HP, WP], F8, kind="ExternalInput").ap()
    xl8 = nc.dram_tensor("xl8p", [BL, C, HP, WP], F8, kind="ExternalInput").ap()
    xb2 = nc.dram_tensor("xb2", [BL, C, H, W], BF16, kind="ExternalInput").ap()
    out4 = nc.dram_tensor("out4", [BL, C, H, W], F32, kind="ExternalOutput").ap()
    rwT = nc.dram_tensor("rwT", [2, 128, 16], F32, kind="ExternalInput").ap()
    rb = nc.dram_tensor("rb", [16, 1], F32, kind="ExternalInput").ap()
    fc1wT = [nc.dram_tensor(f"fc1wTp{n}", [17, 4096], BF16,
                            kind="ExternalInput").ap() for n in (1, 2)]
    w2p = [nc.dram_tensor(f"w2p{n}", [4, 128, 2 * 9 * 256], BF16,
                          kind="ExternalInput").ap() for n in (1, 2)]
    bns = [nc.dram_tensor(f"bns{n}", [2, 128, 1], F32,
                          kind="ExternalInput").ap() for n in (1, 2)]
    bnb1 = nc.dram_tensor("bnb1", [2, 128, 1], F32, kind="ExternalInput").ap()

    with tile.TileContext(nc) as tc:
        build_body(tc, xh8, xl8, xb2, out4, rwT, rb, fc1wT, w2p, bns,
                   bnb1)

    nc.compile()
    return nc


def build_body(tc, xh8, xl8, xb2, out4, rwT, rb, fc1wT, w2p, bns, bnb1):
    nc = tc.nc
    from contextlib import ExitStack
    ctx = ExitStack()

    cpool = ctx.enter_context(tc.tile_pool(name="consts", bufs=1))
    wbf_p = ctx.enter_context(tc.tile_pool(name="wbf", bufs=1))
    wtmp_p = ctx.enter_context(tc.tile_pool(name="wtmp", bufs=1))
    w8_p = ctx.enter_context(tc.tile_pool(name="w8", bufs=4))
    aexp_p = ctx.enter_context(tc.tile_pool(name="aexp", bufs=2))
    small_p = ctx.enter_context(tc.tile_pool(name="small", bufs=4))
    stage_p = ctx.enter_context(tc.tile_pool(name="stage", bufs=4))
    psum_p = ctx.enter_context(tc.tile_pool(name="psum", bufs=7, space="PSUM"))
    psmall_p = ctx.enter_context(tc.tile_pool(name="psmall", bufs=1,
                                              space="PSUM"))
    dram_p = ctx.enter_context(tc.tile_pool(name="dscratch", bufs=2,
                                            space="DRAM"))

    xh_r = xh8.rearrange("s (c p) h w -> s p c (h w)", c=2)
    xl_r0 = xl8.rearrange("s (c p) h w -> s p c (h w)", c=2)

    # ACT table warmup: trigger every LoadActFuncSet at t~0, off the x path
    warm = cpool.tile([1, 1], F32, tag="warm")
    nc.gpsimd.memset(warm[:], 0.0)
    for fn in (AF.Copy, AF.Identity, AF.Relu, AF.Sigmoid):
        nc.scalar.activation(warm[:], warm[:], fn)

    # sample-0 image DMAs lead the queue: they head the gap->fc1->wgen chain
    xpad0h = cpool.tile([128, 2, NPPAD], F8, tag="xpadh0")
    xpad0l = cpool.tile([128, 2, NPPAD], F8, tag="xpadl0")
    for c in range(2):
        nc.sync.dma_start(xpad0h[:, c], xh_r[0, :, c])
    for c in range(2):
        nc.sync.dma_start(xpad0l[:, c], xl_r0[0, :, c])

    # ---- resident constants ----
    rwT_sb = []
    for c in range(2):
        t = cpool.tile([128, 16], F32, tag=f"rwT{c}")
        nc.sync.dma_start(t[:], rwT[c])
        rwT_sb.append(t)
    rb_sb = cpool.tile([16, 1], F32, tag="rb")
    nc.sync.dma_start(rb_sb[:], rb)

    fc1wT_sb = []
    for n in range(2):
        t = cpool.tile([17, 4096], BF16, tag=f"fc1wT{n}")
        if n == 0:
            nc.sync.dma_start(t[:], fc1wT[n])
        fc1wT_sb.append(t)

    w2sb = []   # [n][i] -> [128, 2*9*256] bf16, layout (ch, k, co)
    for n in range(2):
        per = []
        for i in range(4):
            t = cpool.tile([128, 2 * 9 * 256], BF16, tag=f"w2sb{n}{i}")
            per.append(t)
        w2sb.append(per)

    NWQ = 4   # chunks per basis-tile DMA: keeps DMA_ENGINES holds short
    def load_w2sb(n):
        q = 4608 // NWQ
        for i in range(4):
            for j in range(NWQ):
                nc.sync.dma_start(w2sb[n][i][:, q * j:q * (j + 1)],
                                  w2p[n][i][:, q * j:q * (j + 1)])

    bns_sb = []
    for n in range(2):
        per = []
        for cc in range(2):
            t = cpool.tile([128, 1], F32, tag=f"bns{n}{cc}")
            per.append(t)
        bns_sb.append(per)
    bnb1_sb = []
    for cc in range(2):
        t = cpool.tile([128, 1], F32, tag=f"bnb1{cc}")
        bnb1_sb.append(t)

    def load_bn_consts():
        for n in range(2):
            for cc in range(2):
                nc.sync.dma_start(bns_sb[n][cc][:], bns[n][cc])
        for cc in range(2):
            nc.sync.dma_start(bnb1_sb[cc][:], bnb1[cc])

    def load_deferred_consts():
        nc.sync.dma_start(fc1wT_sb[1][:], fc1wT[1])

    gap16 = cpool.tile([17, BL], BF16, tag="gap16")
    nc.gpsimd.memset(gap16[:], 1.0)
    accsink = cpool.tile([128, NPPAD], F8, tag="accsink")

    # persistent padded tiles (double-buffered by sample parity);
    # borders zeroed once here, only interiors are rewritten per sample.
    def make_pads(name, nbuf):
        tiles = []
        for j in range(nbuf):
            t = cpool.tile([128, 2, NPPAD], F8, tag=f"{name}{j}")
            r = t[:].rearrange("p c (h w) -> p c h w", h=HP)
            nc.gpsimd.memset(r[:, :, 0, :], 0.0)
            nc.gpsimd.memset(r[:, :, HP - 1, :], 0.0)
            nc.gpsimd.memset(r[:, :, 1:HP - 1, 0:1], 0.0)
            nc.gpsimd.memset(r[:, :, 1:HP - 1, WP - 1:WP], 0.0)
            tiles.append(t)
        return tiles

    xpad1h = cpool.tile([128, 2, NPPAD], F8, tag="xpadh1")
    xpad1l = cpool.tile([128, 2, NPPAD], F8, tag="xpadl1")
    xpadh = [xpad0h, xpad1h]
    xpadl = [xpad0l, xpad1l]
    o1padh = make_pads("o1padh", 1)[0]
    o1padl = make_pads("o1padl", 1)[0]

    xl_r = xl8.rearrange("s (c p) h w -> s p c (h w)", c=2)

    def load_x_a(s):
        """direct DMA of pre-padded fp8 hi/lo images; gap sums via DVE."""
        j = s % 2
        gsum = []
        if s > 0:
            for c in range(2):
                nc.sync.dma_start(xpadh[j][:, c], xh_r[s, :, c])
        for c in range(2):
            g = small_p.tile([128, 1], F32, tag="gsum")
            if s < 2:
                # prologue: keep DVE free for the first weight chains
                nc.scalar.activation(accsink[:], xpadh[j][:, c],
                                     AF.Copy, accum_out=g[:])
            else:
                nc.vector.tensor_reduce(g[:], xpadh[j][:, c],
                                        mybir.AxisListType.X, ALU.add)
            gsum.append(g)
        if s > 0:
            for c in range(2):
                nc.sync.dma_start(xpadl[j][:, c], xl_r[s, :, c])
        return gsum

    def load_x_b(s, gsum):
        gpt = psmall_p.tile([128, 33], F32, tag="avec_ps")
        gps = gpt[0:16, 32:33]
        ng = len(gsum)
        for c in range(ng):
            nc.tensor.matmul(gps, rwT_sb[c * 2 // ng][:], gsum[c][:],
                             start=(c == 0), stop=(c == ng - 1))
        nc.scalar.activation(gap16[0:16, s:s + 1], gps, AF.Identity,
                             bias=rb_sb[:], scale=1.0)

    def gen_weights_a(wn, s):
        """a = sigmoid(fc1p(gap)); broadcast to aexp via DRAM roundtrip."""
        apt = psmall_p.tile([128, 33], F32, tag="avec_ps")
        aps = apt[:, 0:32]
        for jj in range(32):
            nc.tensor.matmul(aps[:, jj:jj + 1],
                             fc1wT_sb[wn][:, 128 * jj:128 * (jj + 1)],
                             gap16[:, s:s + 1],
                             start=True, stop=True)
        avec = small_p.tile([128, 32], BF16, tag="avec")
        nc.scalar.activation(avec[:], aps, AF.Sigmoid)
        avd = dram_p.tile([4096], BF16, tag="avd")
        # broadcast to [128, (ch, i, co)]: per 64-partition block one
        # contiguous 2048-element replicated read; write/read halves are
        # pipelined (aexp half hp depends only on avd half hp)
        aexp = aexp_p.tile([128, 2 * 4 * 256], BF16, tag="aexp")
        avr = avd[:].rearrange("(hp f) -> hp f", hp=2)
        for hp in range(2):
            nc.scalar.dma_start(
                avr[hp].rearrange("(j p) -> p j", p=128),
                avec[:, 16 * hp:16 * (hp + 1)])
            nc.scalar.dma_start(aexp[64 * hp:64 * (hp + 1), :],
                              avr[hp].unsqueeze(0).broadcast_to([64, 2048]))
        return aexp

    def gen_weights_b(wn, aexp, fast=False):
        """W = sum_i a_i*w2_i (bf16), split into fp8 hi/lo."""
        ae4 = aexp[:].rearrange("p (c i o) -> p c i o", c=2, i=4)
        wv = lambda t: t[:].rearrange("p (c k o) -> p c k o", c=2, k=9)

        def abid(i):
            return (ae4[:, :, i, :].unsqueeze(2)
                    .broadcast_to([128, 2, 9, 256]))

        wbf = wbf_p.tile([128, 2 * 9 * 256], BF16, tag="wbf")
        nc.vector.tensor_mul(wv(wbf), wv(w2sb[wn][0]), abid(0))
        for i in range(1, 4):
            tmp = wtmp_p.tile([128, 2 * 9 * 256], BF16, tag="wtmp")
            nc.vector.tensor_mul(wv(tmp), wv(w2sb[wn][i]), abid(i))
            nc.vector.tensor_add(wbf[:], wbf[:], tmp[:])
        wh = w8_p.tile([128, 2 * 9 * 256], F8, tag="wh")
        heng = nc.vector if fast else nc.gpsimd
        heng.tensor_copy(wh[:], wbf[:])
        wl = w8_p.tile([128, 2 * 9 * 256], F8, tag="wl")
        nc.vector.tensor_sub(wl[:], wbf[:], wh[:])
        return wh, wl

    def conv(wh, wl, srch, srcl, sink, lh_first=False):
        """3-pass fp8 DoubleRow 3x3 conv; sink(cc, t, psum_tile) evacuates."""
        whv = wh[:].rearrange("p (c k o) -> p c k o", c=2, k=9)
        wlv = wl[:].rearrange("p (c k o) -> p c k o", c=2, k=9)
        sh = srch[:].rearrange("p c (h w) -> p c h w", h=HP)
        sl = srcl[:].rearrange("p c (h w) -> p c h w", h=HP)
        passes = (((wlv, sh), (whv, sh), (whv, sl)) if lh_first
                  else ((whv, sh), (whv, sl), (wlv, sh)))
        for cc in range(2):
            for t in range(NT):
                ps = psum_p.tile([128, NFREE], F32, tag="cps")
                idx = 0
                for kg in range(3):
                    for wop, xop in passes:
                        for kk in range(3):
                            k = 3 * kg + kk
                            kh, kw = divmod(k, 3)
                            r0 = TROWS * t + kh
                            nc.tensor.matmul(
                                ps[:],
                                wop[:, :, k, 128 * cc:128 * (cc + 1)],
                                xop[:, :, r0:r0 + TROWS, kw:kw + W],
                                start=(idx == 0), stop=(idx == 26),
                                perf_mode=DR)
                            idx += 1
                sink(cc, t, ps)

    # ---------------- prologue ----------------
    g0 = load_x_a(0)
    load_w2sb(0)
    load_x_b(0, g0)
    ax = gen_weights_a(0, 0)
    load_bn_consts()
    w1 = gen_weights_b(0, ax, fast=True)
    gsum_next = load_x_a(1)
    load_deferred_consts()
    load_w2sb(1)
    ax = gen_weights_a(1, 0)
    w2 = gen_weights_b(1, ax)

    for s in range(BL):
        j = s % 2
        # generate next sample's weights one full iteration ahead
        if s + 1 < BL:
            load_x_b(s + 1, gsum_next)
            ax1 = gen_weights_a(0, s + 1)
            ax2 = gen_weights_a(1, s + 1)
            w1_next = gen_weights_b(0, ax1)

        # ---- conv1 + bn1(+*SO1) + relu -> o1 hi/lo (fp8, padded) ----
        oph = o1padh[:].rearrange("p c (h w) -> p c h w", h=HP)
        opl = o1padl[:].rearrange("p c (h w) -> p c h w", h=HP)

        def sink1(cc, t, ps):
            rows = slice(TROWS * t + 1, TROWS * t + 1 + TROWS)
            psv = ps[:].rearrange("p (h w) -> p h w", h=TROWS)
            nc.scalar.activation(oph[:, cc, rows, 1:1 + W], psv,
                                 AF.Relu, bias=bnb1_sb[cc][:],
                                 scale=bns_sb[0][cc][:])
            obf = stage_p.tile([128, TROWS, W], BF16, tag="o1bf")
            nc.scalar.activation(obf[:], psv, AF.Relu, bias=bnb1_sb[cc][:],
                                 scale=bns_sb[0][cc][:])
            nc.vector.tensor_sub(opl[:, cc, rows, 1:1 + W], obf[:],
                                 oph[:, cc, rows, 1:1 + W])

        conv(w1[0], w1[1], xpadh[j], xpadl[j], sink1)

        if s + 2 < BL:
            gsum_next = load_x_a(s + 2)
        if s + 1 < BL:
            w2_next = gen_weights_b(1, ax2)

        # ---- conv2 + bn2 + residual + relu -> out ----
        def sink2(cc, t, ps):
            t2 = stage_p.tile([128, NFREE], F32, tag="t2")
            xres = stage_p.tile([128, NFREE], BF16, tag="xres")
            xflat = xb2[s, 128 * cc:128 * (cc + 1)].rearrange(
                "c h w -> c (h w)")
            nc.sync.dma_start(xres[:], xflat[:, NFREE * t:NFREE * (t + 1)])
            if s == BL - 1:
                # tail: fused psum evac + residual on DVE (shortest drain)
                nc.vector.scalar_tensor_tensor(t2[:], ps[:], bns_sb[1][cc][:],
                                               xres[:], ALU.mult, ALU.add)
                nc.vector.tensor_scalar_max(t2[:], t2[:], 0.0)
            else:
                nc.scalar.activation(t2[:], ps[:], AF.Identity,
                                     scale=bns_sb[1][cc][:])
                nc.gpsimd.tensor_add(t2[:], t2[:], xres[:])
                nc.gpsimd.tensor_scalar_max(t2[:], t2[:], 0.0)
            oflat = out4[s, 128 * cc:128 * (cc + 1)].rearrange(
                "c h w -> c (h w)")
            nc.sync.dma_start(oflat[:, NFREE * t:NFREE * (t + 1)], t2[:])

        conv(w2[0], w2[1], o1padh, o1padl, sink2)
        if s + 1 < BL:
            w1 = w1_next
            w2 = w2_next

    ctx.close()


_NC_CACHE = {}


def get_program():
    if "nc" not in _NC_CACHE:
        _NC_CACHE["nc"] = build_program()
    return _NC_CACHE["nc"]


def prep_inputs(inputs):
    f32 = lambda a: np.ascontiguousarray(np.asarray(a, np.float32))
    bf = lambda a: np.ascontiguousarray(
        np.asarray(a, np.float32).astype(BFNP))

    x = np.asarray(inputs["x"], np.float32)

    # fp8 hi/lo split of x*SX (exact residual, same scale); both pre-padded
    xs = x * SX
    xh = np.zeros((B, C, HP, WP), E4NP)
    xh[:, :, 1:1 + H, 1:1 + W] = xs.astype(E4NP)
    xl = np.zeros((B, C, HP, WP), E4NP)
    xl[:, :, 1:1 + H, 1:1 + W] = (xs - xh[:, :, 1:1 + H, 1:1 + W]
                                  .astype(np.float32)).astype(E4NP)

    def perm_fc1():
        n = np.arange(4096)
        return (16 * (n % 256) + 8 * ((n // 1024) % 2) + 4 * (n // 2048)
                + (n // 256) % 4)

    PI = perm_fc1()

    def pack_fc1(fc1_w, fc1_b):
        wT = np.asarray(fc1_w, np.float32).T      # [16, 4096]
        aug = np.concatenate([wT, np.asarray(fc1_b, np.float32)[None, :]],
                             axis=0)              # [17, 4096]
        return bf(aug[:, PI])

    def pack_w2(fc2_w):
        w2 = np.asarray(fc2_w, np.float32).reshape(1024, 576, 4) * SW
        p = np.arange(128)
        ch = np.arange(2)
        k = np.arange(9)
        co = np.arange(256)
        # [p, ch, k, co]
        g = (co[None, None, None, :] * 4 + 2 * ch[None, :, None, None]
             + (p[:, None, None, None] // 64))
        o = (p[:, None, None, None] % 64) * 9 + k[None, None, :, None]
        out = np.empty((4, 128, 2, 9, 256), np.float32)
        for i in range(4):
            out[i] = w2[g, o, i]
        return bf(out.reshape(4, 128, 2 * 9 * 256))

    def bn_fold(g, b, m, v):
        sc = np.asarray(g, np.float32) / np.sqrt(np.asarray(v, np.float32) + EPS)
        bia = np.asarray(b, np.float32) - np.asarray(m, np.float32) * sc
        return sc, bia

    sc1, bia1 = bn_fold(inputs["bn1_g"], inputs["bn1_b"], inputs["bn1_m"],
                        inputs["bn1_v"])
    sc2, bia2 = bn_fold(inputs["bn2_g"], inputs["bn2_b"], inputs["bn2_m"],
                        inputs["bn2_v"])

    fc1w1 = pack_fc1(inputs["w1_fc1_w"], inputs["w1_fc1_b"])
    fc1w2 = pack_fc1(inputs["w2_fc1_w"], inputs["w2_fc1_b"])

    base = {
        "rwT": f32((np.asarray(inputs["reduce_w"], np.float32).T
                    / (NPIX * SX)).reshape(2, 128, 16)),
        "rb": f32(np.asarray(inputs["reduce_b"]).reshape(16, 1)),
        "fc1wTp1": fc1w1, "fc1wTp2": fc1w2,
        "w2p1": pack_w2(inputs["w1_fc2_w"]),
        "w2p2": pack_w2(inputs["w2_fc2_w"]),
        "bns1": f32((sc1 * SO1 / (SX * SW)).reshape(2, 128, 1)),
        "bnb1": f32((bia1 * SO1).reshape(2, 128, 1)),
        "bns2": f32((sc2 / (SO1 * SW)).reshape(2, 128, 1)),
    }

    # residual with bn2 bias folded in
    xb2 = (x + bia2[None, :, None, None]).astype(BFNP)

    in_maps = []
    for i in range(NCORES):
        m = dict(base)
        sl = slice(i * BL, (i + 1) * BL)
        m["xh8p"] = np.ascontiguousarray(xh[sl])
        m["xl8p"] = np.ascontiguousarray(xl[sl])
        m["xb2"] = np.ascontiguousarray(xb2[sl])
        in_maps.append(m)
    return in_maps


def kernel(**inputs):
    in_maps = prep_inputs(inputs)
    nc = get_program()
    res = bass_utils.run_bass_kernel_spmd(nc, in_maps,
                                          core_ids=list(range(NCORES)))
    out = np.concatenate([r["out4"] for r in res.results], axis=0)
    return out.astype(np.float32)
